# revision 15
# baseline (speedup 1.0000x reference)
"""Trainium2 Bass kernel for nn_ConvEnhanced (conv/attn/quantum fused head).

Reference math per sample (x is (16,) f32, all in [0,1)):
    cls  = sigmoid(dot(x, w) + b)
    attn = mean_j sigmoid(a * x_j)
    q    = mean_j sin^2(pi * x_j / 2)        (threshold/where is a no-op, x >= 0)
    out  = alpha * cls * attn + (1 - alpha) * q

Device strategy (pure data parallel over 8 cores, 524288 samples/core):
  - x cast to fp16 on host (rel err <= 2^-11, inside the 2e-2 gate), shipped
    as (128, 65536) tile-packed j-major: within a device tile of t samples,
    j-block j is a contiguous t-run (matmul rhs stays contiguous; strided
    rhs measures ~5x slower PE streaming).
  - ScalarE: one full tanh pass th = tanh((a/2)x) (sigmoid(ax) =
    .5+.5tanh(ax/2)); a SMALL Sin pass (K_ACT j-blocks) cs = sin(pi/2-pi*x)
    = cos(pi*x); and the cls tanh on the dot-product sums.
  - DVE: the quantum path for the remaining 16-K_ACT j-blocks runs as ONE
    fused custom-DVE op per j-block (8-deep ALU pipeline, ~1.1 cyc/elem,
    validated on HW):
        SINSQ_INIT: qacc  = P5(x)^2          (j = first DVE block)
        SINSQ_ACC:  qacc += P5(x)^2          (remaining blocks, in-place)
    where P5(x) = x*(B0 + B1 x^2 + B2 x^4) ~ sin((pi/2)x), coefficients
    minimax-fit on |P5^2 - sin^2| (max err 9.8e-5). This replaces both the
    5-op elementwise polynomial AND the PE reduction of the baseline: qacc
    accumulates per-sample sums directly.
  - TensorE: segmented sums as PSUM-accumulating N=t matmuls per j-block:
        S_wx  += diag(w_j) @ x[:, j-block]     (fp16 in, fp32 accum)
        S_th  += I @ th[:, j-block]            (16 blocks)
        S_cs  += I @ cs[:, j-block]            (K_ACT blocks)
  - Tail (DVE, pipelined one tile behind): with tc = tanh(.5 S_wx + b/2),
        out = (1+tc)*(alpha/4 + alpha/64*S_th)            [custom TAIL1]
            + (-(1-alpha)/32)*S_cs + ...                  [stt]
            + ((1-alpha)/16)*qacc + const                 [AFFINE_THEN_ADD]
    out is written fp16 (values in (0,1)) and upcast on the host.
"""

import numpy as np

try:
    import concourse.bass as bass  # noqa: F401
except ImportError:  # pragma: no cover
    import sys

    sys.path.insert(0, "/opt/trn_rl_repo")
    import concourse.bass as bass  # noqa: F401

import concourse.dve_ops as dve_ops
from concourse.dve_ops import DveOp
from concourse.dve_spec import (
    C0,
    C1,
    C2,
    One,
    Spec,
    Src0,
    Src1,
    lower as dve_lower,
    sq,
)
from concourse.dve_spec import _has_src1 as has_src1
from concourse.dve_uop import DveOpSpec

B = 4_194_304  # total samples
N_CORES = 8
P = 128  # partitions
KE = 16  # elements per sample (4x4 patch)
B_LOC = B // N_CORES  # samples per core
SPP = B_LOC // P  # samples per partition (4096)

# j-blocks evaluated on ScalarE's Sin table (cos(pi x) values, PE-reduced);
# the other KE-K_ACT j-blocks run on the fused DVE sin^2 accumulator ops.
K_ACT = 2

# sin((pi/2)x) ~ x*(B0 + B1 x^2 + B2 x^4): minimax fit of |P^2 - sin^2|
# over x in [0,1], max err 9.8e-5.
SIN_B0 = 1.57009095
SIN_B1 = -0.64138591
SIN_B2 = 0.07134415

# Per-device-tile sample counts (per partition). Ramp-up tiles keep the
# compute engines fed while the x DMA streams in (~12us per 1024-sample
# tile); deep middle tiles amortize per-instruction overhead (custom-DVE
# op carries ~130ns fixed cost). Sum must equal SPP.
TILES = (128, 256, 384, 512, 768, 1024, 768, 256)

# Run the per-sample tail combine on the (otherwise idle) gpsimd engine
# instead of DVE. gpsimd shares the DVE's SBUF port, so this trades ~15us
# of DVE time against possible port-contention slowdown — A/B measured.
TAIL_ON_POOL = False

_NC_CACHE = {}


def _register_op(name, spec, subdim=False):
    """Register a custom DVE op into the dve_ops tables (idempotent)."""
    if name in dve_ops._SUB_OPCODE_FOR_NAME:
        return next(o for o in dve_ops.OPS if o.name == name)
    row = dve_ops._CUSTOM_DVE_ROW_BASE + len(dve_ops.OPS)
    assert row < 0x20, "custom-DVE opcode rows exhausted"
    shas = {}
    for ver in ("v3", "v4"):
        so = DveOpSpec(
            name=name, opcode=row, uops=dve_lower(spec, ver=ver),
            rd1_en=has_src1(spec),
        )
        shas[ver] = so.sha(ver)
    op = DveOp(name, spec, subdim=subdim, uops_sha=shas)
    dve_ops.OPS.append(op)
    dve_ops._SUB_OPCODE_FOR_NAME[name] = row
    dve_ops.CUSTOM_DVE_SPECS[name] = spec
    return op


def _p5sq(x):
    v = x * x
    p = ((v * SIN_B2 + SIN_B1) * v + SIN_B0) * x
    return p * p


def _sinsq_init_ref(in0, in1, c0, c1, c2):
    return _p5sq(in0.astype(np.float32)).astype(np.float32)


def _sinsq_acc_ref(in0, in1, c0, c1, c2):
    return (_p5sq(in0.astype(np.float32)) + in1.astype(np.float32)).astype(
        np.float32
    )


def _tail1_ref(in0, in1, c0, c1, c2):
    # (tc + 1) * (S_th * c0 + c1)
    return (
        (in0.astype(np.float32) + 1.0) * (in1.astype(np.float32) * c0 + c1)
    ).astype(np.float32)


def _sinsq_initb_ref(in0, in1, c0, c1, c2):
    # in1 is the spilled C3 ([P,1] broadcast): seed the accumulator with it
    return (_p5sq(in0.astype(np.float32)) + in1.astype(np.float32)).astype(
        np.float32
    )


_v = sq(Src0)
_p5 = ((_v * C0 + C1) * _v + C2) * Src0
SINSQ_INIT = _register_op(
    "NNCE_SINSQ_INIT", Spec(body=sq(_p5), reference=_sinsq_init_ref)
)
from concourse.dve_spec import C3, _spill_c3_to_src1  # noqa: E402

SINSQ_INITB = _register_op(
    "NNCE_SINSQ_INITB",
    Spec(body=_spill_c3_to_src1(sq(_p5) + C3), reference=_sinsq_initb_ref),
)
SINSQ_ACC = _register_op(
    "NNCE_SINSQ_ACC", Spec(body=sq(_p5) + Src1, reference=_sinsq_acc_ref)
)
TAIL1 = _register_op(
    "NNCE_TAIL1",
    Spec(body=(Src0 + One) * (Src1 * C0 + C1), reference=_tail1_ref),
)


def _build(spp, tiles, k_act=K_ACT, tail_on_pool=None):
    if tail_on_pool is None:
        tail_on_pool = TAIL_ON_POOL
    """Build the Bass/Tile program for one core (SPMD: identical on all)."""
    import concourse.bacc as bacc
    import concourse.tile as tile
    from concourse import mybir

    F32 = mybir.dt.float32
    F16 = mybir.dt.float16
    A = mybir.ActivationFunctionType
    Op = mybir.AluOpType

    tiles = list(tiles)
    assert sum(tiles) == spp
    t_max = max(tiles)
    ft_max = KE * t_max
    k_dve = KE - k_act  # j-blocks on the DVE sin^2 path (0..k_dve-1)
    # PE/PSUM granularity: a matmul dest must fit one PSUM bank (512 f32).
    H = 512

    PI = float(np.pi)

    nc = bacc.Bacc("TRN2", target_bir_lowering=False)
    x_d = nc.declare_dram_parameter("x", [P, spp * KE], F16, isOutput=False)
    wd_d = nc.declare_dram_parameter("wdiag", [P, KE * P], F16, isOutput=False)
    id_d = nc.declare_dram_parameter("ident", [P, P], F16, isOutput=False)
    c_d = nc.declare_dram_parameter("consts", [P, 12], F32, isOutput=False)
    o_d = nc.declare_dram_parameter("out", [P, spp], F16, isOutput=True)

    with tile.TileContext(nc) as tc:
        with (
            tc.tile_pool(name="const", bufs=1) as cpool,
            tc.tile_pool(name="xp", bufs=2) as xpool,
            tc.tile_pool(name="thp", bufs=2) as thpool,
            tc.tile_pool(name="csp", bufs=2) as cspool,
            tc.tile_pool(name="qp", bufs=2) as qpool,
            tc.tile_pool(name="tcp", bufs=2) as tcpool,
            tc.tile_pool(name="tlp", bufs=2) as tlpool,
            tc.tile_pool(name="op", bufs=2) as opool,
            tc.tile_pool(name="pwx", bufs=2, space="PSUM") as wxpool,
            tc.tile_pool(name="pth", bufs=2, space="PSUM") as thppool,
            tc.tile_pool(name="pcs", bufs=2, space="PSUM") as csppool,
        ):
            # Consts ride the sync HWDGE queue. Order: consts, tile-0 x
            # (first-ACT critical path), then wd/id (needed once mms start).
            c_sb = cpool.tile([P, 12], F32, tag="c")
            nc.sync.dma_start(c_sb[:], c_d[:])
            x_first = xpool.tile([P, ft_max], F16, tag="x")
            nc.sync.dma_start(
                x_first[:, 0 : KE * tiles[0]], x_d[:, 0 : KE * tiles[0]]
            )
            wd_sb = cpool.tile([P, KE * P], F16, tag="wd")
            nc.sync.dma_start(wd_sb[:], wd_d[:])
            id_sb = cpool.tile([P, P], F16, tag="id")
            nc.sync.dma_start(id_sb[:], id_d[:])

            # Dummy 1-elem ACT: force the single pinned table set (holds
            # both Tanh and Sin) to load behind the tile-0 x DMA.
            warm_i = cpool.tile([P, 1], F32, tag="warm_i")
            nc.gpsimd.memset(warm_i[:], 0.0)
            warm_o = cpool.tile([P, 1], F32, tag="warm_o")
            nc.scalar.activation(warm_o[:], warm_i[:], A.Tanh)

            wd_v = wd_sb[:].rearrange("p (j m) -> p j m", j=KE)

            def emit_tail(st):
                """Combine + store for a finished 512-half (one unit behind)."""
                h_len, off, ps_th, ps_cs, qacc, q0, tc_t = st
                if tail_on_pool:
                    # gpsimd variant: 4 stock ops; the additive constant c_0
                    # rides the qacc seed (SINSQ_INITB's C3), so every op
                    # here fits a two-scalar/stt shape.
                    u = tlpool.tile([P, H], F32, tag="u")
                    nc.gpsimd.tensor_scalar(
                        u[:, 0:h_len], ps_th[:, 0:h_len],
                        c_sb[:, 2:3], c_sb[:, 3:4], Op.mult, Op.add,
                    )
                    p1 = tlpool.tile([P, H], F32, tag="p1")
                    nc.gpsimd.scalar_tensor_tensor(
                        p1[:, 0:h_len], tc_t[:, 0:h_len], 1.0,
                        u[:, 0:h_len], Op.add, Op.mult,
                    )
                    if k_act > 0:
                        p2 = tlpool.tile([P, H], F32, tag="p2")
                        nc.gpsimd.scalar_tensor_tensor(
                            p2[:, 0:h_len], ps_cs[:, 0:h_len], c_sb[:, 4:5],
                            p1[:, 0:h_len], Op.mult, Op.add,
                        )
                    else:
                        p2 = p1
                    o_t = opool.tile([P, H], F16, tag="o")
                    nc.gpsimd.scalar_tensor_tensor(
                        o_t[:, 0:h_len], qacc[:, q0 : q0 + h_len],
                        c_sb[:, 5:6], p2[:, 0:h_len], Op.mult, Op.add,
                    )
                else:
                    # p1 = (tc+1) * (c_thm*S_th + c_tha)
                    p1 = tlpool.tile([P, H], F32, tag="p1")
                    nc.vector._custom_dve(
                        TAIL1,
                        out=p1[:, 0:h_len],
                        in0=tc_t[:, 0:h_len],
                        in1=ps_th[:, 0:h_len],
                        s0=c_sb[:, 2:3],
                        s1=c_sb[:, 3:4],
                    )
                    if k_act > 0:
                        # p2 = c_cs*S_cs + p1
                        p2 = tlpool.tile([P, H], F32, tag="p2")
                        nc.vector.scalar_tensor_tensor(
                            p2[:, 0:h_len], ps_cs[:, 0:h_len], c_sb[:, 4:5],
                            p1[:, 0:h_len], Op.mult, Op.add,
                        )
                    else:
                        p2 = p1
                    # out = c_q*qacc + p2   (fp16 out; c_0 rides the qacc seed)
                    o_t = opool.tile([P, H], F16, tag="o")
                    nc.vector.scalar_tensor_tensor(
                        o_t[:, 0:h_len], qacc[:, q0 : q0 + h_len],
                        c_sb[:, 5:6], p2[:, 0:h_len], Op.mult, Op.add,
                    )
                nc.sync.dma_start(o_d[:, off : off + h_len], o_t[:, 0:h_len])

            pending = None
            off = 0
            for t_idx, t_tile in enumerate(tiles):
                ft = KE * t_tile
                e0 = off * KE
                if t_idx == 0:
                    x_t = x_first
                else:
                    x_t = xpool.tile([P, ft_max], F16, tag="x")
                    nc.sync.dma_start(x_t[:, 0:ft], x_d[:, e0 : e0 + ft])

                # th = tanh((a/2) x) over all KE j-blocks
                th_t = thpool.tile([P, ft_max], F16, tag="th")
                nc.scalar.activation(
                    th_t[:, 0:ft], x_t[:, 0:ft], A.Tanh, scale=c_sb[:, 0:1]
                )
                # cs = sin(pi/2 - pi x) = cos(pi x) on the last k_act blocks
                if k_act > 0:
                    cs_t = cspool.tile([P, k_act * t_max], F16, tag="cs")
                    nc.scalar.activation(
                        cs_t[:, 0 : k_act * t_tile],
                        x_t[:, k_dve * t_tile : ft],
                        A.Sin,
                        bias=c_sb[:, 7:8],
                        scale=-PI,
                    )

                # Fused DVE sin^2 accumulation over j-blocks 0..k_dve-1;
                # the qacc seed carries the tail's additive constant c_0/c_q.
                qacc = qpool.tile([P, t_max], F32, tag="qacc")
                nc.vector._custom_dve(
                    SINSQ_INITB,
                    out=qacc[:, 0:t_tile],
                    in0=x_t[:, 0:t_tile],
                    in1=c_sb[:, 6:7],
                    s0=SIN_B2, s1=SIN_B1, imm2=SIN_B0,
                )
                for j in range(1, k_dve):
                    nc.vector._custom_dve(
                        SINSQ_ACC,
                        out=qacc[:, 0:t_tile],
                        in0=x_t[:, bass.ts(j, t_tile)],
                        in1=qacc[:, 0:t_tile],
                        s0=SIN_B2, s1=SIN_B1, imm2=SIN_B0,
                    )

                # PE reductions + tc + tail per 512-column half (PSUM bank)
                for h0 in range(0, t_tile, H):
                    h_len = min(H, t_tile - h0)
                    ps_wx = wxpool.tile([P, H], F32, tag="pwx")
                    ps_th = thppool.tile([P, H], F32, tag="pth")
                    if k_act:
                        ps_cs = csppool.tile([P, H], F32, tag="pcs")
                    else:
                        ps_cs = None

                    for j in range(KE):
                        nc.tensor.matmul(
                            ps_wx[:, 0:h_len],
                            lhsT=wd_v[:, j, :],
                            rhs=x_t[:, j * t_tile + h0 : j * t_tile + h0 + h_len],
                            start=(j == 0),
                            stop=(j == KE - 1),
                        )
                    for j in range(KE):
                        nc.tensor.matmul(
                            ps_th[:, 0:h_len],
                            lhsT=id_sb[:],
                            rhs=th_t[:, j * t_tile + h0 : j * t_tile + h0 + h_len],
                            start=(j == 0),
                            stop=(j == KE - 1),
                        )
                    for j in range(k_act):
                        nc.tensor.matmul(
                            ps_cs[:, 0:h_len],
                            lhsT=id_sb[:],
                            rhs=cs_t[:, j * t_tile + h0 : j * t_tile + h0 + h_len],
                            start=(j == 0),
                            stop=(j == k_act - 1),
                        )

                    # tc = tanh(0.5*S_wx + b/2)
                    tc_t = tcpool.tile([P, H], F32, tag="tc")
                    nc.scalar.activation(
                        tc_t[:, 0:h_len], ps_wx[:, 0:h_len], A.Tanh,
                        bias=c_sb[:, 1:2], scale=0.5,
                    )
                    if pending is not None:
                        emit_tail(pending)
                    pending = (h_len, off + h0, ps_th, ps_cs, qacc, h0, tc_t)
                off += t_tile
            emit_tail(pending)

    # Pin Tanh+Sin to the one table set holding both (silu_and_others) so
    # the act-table pass emits a single load instead of per-tile switches.
    import concourse.bacc as bacc
    import concourse.hw_specs as hw_specs

    _orig_gat = hw_specs.get_activation_tables

    def _pinned_tables(arch):
        tabs = {k: set(v) for k, v in _orig_gat(arch).items()}
        assert A.Tanh in tabs["silu_and_others"] and A.Sin in tabs["silu_and_others"]
        for name, fns in tabs.items():
            if name != "silu_and_others":
                fns.discard(A.Tanh)
                fns.discard(A.Sin)
        return tabs

    bacc.get_activation_tables = _pinned_tables
    try:
        nc.compile()
    finally:
        bacc.get_activation_tables = _orig_gat
    return nc


def get_nc(spp=SPP, tiles=None):
    if tiles is None:
        tiles = TILES
    key = (spp, tuple(tiles))
    if key not in _NC_CACHE:
        _NC_CACHE[key] = _build(spp, tiles)
    return _NC_CACHE[key]


def make_const_inputs(conv_w, conv_b, attn_w, alpha):
    """Host-side packing of the tiny runtime parameters."""
    w = np.asarray(conv_w, dtype=np.float32).reshape(KE)
    b = float(np.asarray(conv_b, np.float32).reshape(-1)[0])
    a = float(np.asarray(attn_w, np.float32).reshape(-1)[0])
    al = float(np.asarray(alpha, np.float32))

    wdiag = np.zeros((P, KE, P), dtype=np.float16)
    idx = np.arange(P)
    wdiag[idx, :, idx] = w[None, :].astype(np.float16)
    wdiag = np.ascontiguousarray(wdiag.reshape(P, KE * P))

    ident = np.ascontiguousarray(np.eye(P, dtype=np.float16))

    row = np.zeros(12, dtype=np.float32)
    row[0] = a / 2.0  # scale for tanh(a x / 2)
    row[1] = b / 2.0  # bias for tanh(0.5 S_wx + b/2)
    row[2] = al / 64.0  # TAIL1 c0 (S_th coeff)
    row[3] = al / 4.0  # TAIL1 c1
    row[4] = -(1.0 - al) / 32.0  # S_cs coeff
    row[5] = (1.0 - al) / 16.0  # qacc coeff
    row[6] = K_ACT / 2.0  # qacc seed: tail const (1-al)/32*K_ACT over row[5]
    row[7] = np.pi / 2.0  # bias for sin(pi/2 - pi x)
    consts = np.ascontiguousarray(np.tile(row[None, :], (P, 1)))
    return wdiag, ident, consts


def pack_x(x3d, tiles):
    """[..., spp, KE] f32 -> [..., spp*KE] fp16, tile-packed element-major."""
    *lead, spp, ke = x3d.shape
    assert sum(tiles) == spp
    v = x3d.astype(np.float16)
    out = np.empty((*lead, spp * ke), dtype=np.float16)
    off = 0
    for t in tiles:
        seg = np.swapaxes(v[..., off : off + t, :], -1, -2)
        out[..., off * ke : (off + t) * ke] = seg.reshape(*lead, t * ke)
        off += t
    return out


def prep_x(x, tiles=None):
    """Cast the f32 input to fp16, shard and tile-pack (cores, P, spp*KE)."""
    if tiles is None:
        tiles = TILES
    x = np.asarray(x)
    assert x.size == B * KE
    return pack_x(x.reshape(N_CORES, P, SPP, KE), tiles)


def kernel(x, conv_w, conv_b, attn_w, alpha):
    from concourse.bass_utils import run_bass_kernel_spmd

    xs = prep_x(x)
    wdiag, ident, consts = make_const_inputs(conv_w, conv_b, attn_w, alpha)

    nc = get_nc()
    in_maps = [
        {"x": xs[c], "wdiag": wdiag, "ident": ident, "consts": consts}
        for c in range(N_CORES)
    ]
    res = run_bass_kernel_spmd(nc, in_maps, list(range(N_CORES)))
    out = np.concatenate(
        [
            np.asarray(res.results[c]["out"]).astype(np.float32).reshape(-1)
            for c in range(N_CORES)
        ]
    )
    return out


# revision 18
# speedup vs baseline: 1.0758x; 1.0758x over previous
"""Trainium2 Bass kernel for nn_ConvEnhanced (conv/attn/quantum fused head).

Reference math per sample (x is (16,) f32, all in [0,1)):
    cls  = sigmoid(dot(x, w) + b)
    attn = mean_j sigmoid(a * x_j)
    q    = mean_j sin^2(pi * x_j / 2)        (threshold/where is a no-op, x >= 0)
    out  = alpha * cls * attn + (1 - alpha) * q

Device strategy (pure data parallel over 8 cores, 524288 samples/core):
  - x cast to fp16 on host (rel err <= 2^-11, inside the 2e-2 gate), shipped
    as (128, 65536) tile-packed j-major: within a device tile of t samples,
    j-block j is a contiguous t-run (matmul rhs stays contiguous; strided
    rhs measures ~5x slower PE streaming).
  - ScalarE: one full tanh pass th = tanh((a/2)x) (sigmoid(ax) =
    .5+.5tanh(ax/2)); a SMALL Sin pass (K_ACT j-blocks) cs = sin(pi/2-pi*x)
    = cos(pi*x); and the cls tanh on the dot-product sums.
  - DVE: the quantum path for the remaining 16-K_ACT j-blocks runs as ONE
    fused custom-DVE op per j-block (8-deep ALU pipeline, ~1.1 cyc/elem,
    validated on HW):
        SINSQ_INIT: qacc  = P5(x)^2          (j = first DVE block)
        SINSQ_ACC:  qacc += P5(x)^2          (remaining blocks, in-place)
    where P5(x) = x*(B0 + B1 x^2 + B2 x^4) ~ sin((pi/2)x), coefficients
    minimax-fit on |P5^2 - sin^2| (max err 9.8e-5). This replaces both the
    5-op elementwise polynomial AND the PE reduction of the baseline: qacc
    accumulates per-sample sums directly.
  - TensorE: segmented sums as PSUM-accumulating N=t matmuls per j-block:
        S_wx  += diag(w_j) @ x[:, j-block]     (fp16 in, fp32 accum)
        S_th  += I @ th[:, j-block]            (16 blocks)
        S_cs  += I @ cs[:, j-block]            (K_ACT blocks)
  - Tail (DVE, pipelined one tile behind): with tc = tanh(.5 S_wx + b/2),
        out = (1+tc)*(alpha/4 + alpha/64*S_th)            [custom TAIL1]
            + (-(1-alpha)/32)*S_cs + ...                  [stt]
            + ((1-alpha)/16)*qacc + const                 [AFFINE_THEN_ADD]
    out is written fp16 (values in (0,1)) and upcast on the host.
"""

import numpy as np

try:
    import concourse.bass as bass  # noqa: F401
except ImportError:  # pragma: no cover
    import sys

    sys.path.insert(0, "/opt/trn_rl_repo")
    import concourse.bass as bass  # noqa: F401

import concourse.dve_ops as dve_ops
from concourse.dve_ops import DveOp
from concourse.dve_spec import (
    C0,
    C1,
    C2,
    One,
    Spec,
    Src0,
    Src1,
    lower as dve_lower,
    sq,
)
from concourse.dve_spec import _has_src1 as has_src1
from concourse.dve_uop import DveOpSpec

B = 4_194_304  # total samples
N_CORES = 8
P = 128  # partitions
KE = 16  # elements per sample (4x4 patch)
B_LOC = B // N_CORES  # samples per core
SPP = B_LOC // P  # samples per partition (4096)

# j-blocks evaluated on ScalarE's Sin table (cos(pi x) values, PE-reduced);
# the other KE-K_ACT j-blocks run on the fused DVE sin^2 accumulator ops.
K_ACT = 2

# sin((pi/2)x) ~ x*(B0 + B1 x^2 + B2 x^4): minimax fit of |P^2 - sin^2|
# over x in [0,1], max err 9.8e-5.
SIN_B0 = 1.57009095
SIN_B1 = -0.64138591
SIN_B2 = 0.07134415

# Per-device-tile sample counts (per partition). Ramp-up tiles keep the
# compute engines fed while the x DMA streams in (~12us per 1024-sample
# tile); deep middle tiles amortize per-instruction overhead (custom-DVE
# op carries ~130ns fixed cost). Sum must equal SPP.
TILES = (512, 1024, 1024, 1024, 512)

# Run the per-sample tail combine on the (otherwise idle) gpsimd engine
# instead of DVE. gpsimd shares the DVE's SBUF port, so this trades ~15us
# of DVE time against possible port-contention slowdown — A/B measured.
TAIL_ON_POOL = False

_NC_CACHE = {}


def _register_op(name, spec, subdim=False):
    """Register a custom DVE op into the dve_ops tables (idempotent)."""
    if name in dve_ops._SUB_OPCODE_FOR_NAME:
        return next(o for o in dve_ops.OPS if o.name == name)
    row = dve_ops._CUSTOM_DVE_ROW_BASE + len(dve_ops.OPS)
    assert row < 0x20, "custom-DVE opcode rows exhausted"
    shas = {}
    for ver in ("v3", "v4"):
        so = DveOpSpec(
            name=name, opcode=row, uops=dve_lower(spec, ver=ver),
            rd1_en=has_src1(spec),
        )
        shas[ver] = so.sha(ver)
    op = DveOp(name, spec, subdim=subdim, uops_sha=shas)
    dve_ops.OPS.append(op)
    dve_ops._SUB_OPCODE_FOR_NAME[name] = row
    dve_ops.CUSTOM_DVE_SPECS[name] = spec
    return op


def _p5sq(x):
    v = x * x
    p = ((v * SIN_B2 + SIN_B1) * v + SIN_B0) * x
    return p * p


def _sinsq_init_ref(in0, in1, c0, c1, c2):
    return _p5sq(in0.astype(np.float32)).astype(np.float32)


def _sinsq_acc_ref(in0, in1, c0, c1, c2):
    return (_p5sq(in0.astype(np.float32)) + in1.astype(np.float32)).astype(
        np.float32
    )


def _tail1_ref(in0, in1, c0, c1, c2):
    # (tc + 1) * (S_th * c0 + c1)
    return (
        (in0.astype(np.float32) + 1.0) * (in1.astype(np.float32) * c0 + c1)
    ).astype(np.float32)


def _sinsq_initb_ref(in0, in1, c0, c1, c2):
    # in1 is the spilled C3 ([P,1] broadcast): seed the accumulator with it
    return (_p5sq(in0.astype(np.float32)) + in1.astype(np.float32)).astype(
        np.float32
    )


_v = sq(Src0)
_p5 = ((_v * C0 + C1) * _v + C2) * Src0
SINSQ_INIT = _register_op(
    "NNCE_SINSQ_INIT", Spec(body=sq(_p5), reference=_sinsq_init_ref)
)
from concourse.dve_spec import C3, _spill_c3_to_src1  # noqa: E402

SINSQ_INITB = _register_op(
    "NNCE_SINSQ_INITB",
    Spec(body=_spill_c3_to_src1(sq(_p5) + C3), reference=_sinsq_initb_ref),
)
SINSQ_ACC = _register_op(
    "NNCE_SINSQ_ACC", Spec(body=sq(_p5) + Src1, reference=_sinsq_acc_ref)
)
TAIL1 = _register_op(
    "NNCE_TAIL1",
    Spec(body=(Src0 + One) * (Src1 * C0 + C1), reference=_tail1_ref),
)


def _build(spp, tiles, k_act=K_ACT, tail_on_pool=None):
    if tail_on_pool is None:
        tail_on_pool = TAIL_ON_POOL
    """Build the Bass/Tile program for one core (SPMD: identical on all)."""
    import concourse.bacc as bacc
    import concourse.tile as tile
    from concourse import mybir

    F32 = mybir.dt.float32
    F16 = mybir.dt.float16
    A = mybir.ActivationFunctionType
    Op = mybir.AluOpType

    tiles = list(tiles)
    assert sum(tiles) == spp
    t_max = max(tiles)
    ft_max = KE * t_max
    k_dve = KE - k_act  # j-blocks on the DVE sin^2 path (0..k_dve-1)
    # PE/PSUM granularity: a matmul dest must fit one PSUM bank (512 f32).
    H = 512

    PI = float(np.pi)

    nc = bacc.Bacc("TRN2", target_bir_lowering=False)
    x_d = nc.declare_dram_parameter("x", [P, spp * KE], F16, isOutput=False)
    wd_d = nc.declare_dram_parameter("wdiag", [P, KE * P], F16, isOutput=False)
    id_d = nc.declare_dram_parameter("ident", [P, P], F16, isOutput=False)
    c_d = nc.declare_dram_parameter("consts", [P, 12], F32, isOutput=False)
    o_d = nc.declare_dram_parameter("out", [P, spp], F16, isOutput=True)

    with tile.TileContext(nc) as tc:
        with (
            tc.tile_pool(name="const", bufs=1) as cpool,
            tc.tile_pool(name="xp", bufs=2) as xpool,
            tc.tile_pool(name="thp", bufs=2) as thpool,
            tc.tile_pool(name="csp", bufs=2) as cspool,
            tc.tile_pool(name="qp", bufs=2) as qpool,
            tc.tile_pool(name="tcp", bufs=2) as tcpool,
            tc.tile_pool(name="tlp", bufs=2) as tlpool,
            tc.tile_pool(name="op", bufs=2) as opool,
            tc.tile_pool(name="pwx", bufs=2, space="PSUM") as wxpool,
            tc.tile_pool(name="pth", bufs=2, space="PSUM") as thppool,
            tc.tile_pool(name="pcs", bufs=2, space="PSUM") as csppool,
        ):
            # Consts ride the sync HWDGE queue. Order: consts, tile-0 x
            # (first-ACT critical path), then wd/id (needed once mms start).
            c_sb = cpool.tile([P, 12], F32, tag="c")
            nc.sync.dma_start(c_sb[:], c_d[:])
            x_first = xpool.tile([P, ft_max], F16, tag="x")
            nc.sync.dma_start(
                x_first[:, 0 : KE * tiles[0]], x_d[:, 0 : KE * tiles[0]]
            )
            wd_sb = cpool.tile([P, KE * P], F16, tag="wd")
            nc.sync.dma_start(wd_sb[:], wd_d[:])
            id_sb = cpool.tile([P, P], F16, tag="id")
            nc.sync.dma_start(id_sb[:], id_d[:])

            # Dummy 1-elem ACT: force the single pinned table set (holds
            # both Tanh and Sin) to load behind the tile-0 x DMA.
            warm_i = cpool.tile([P, 1], F32, tag="warm_i")
            nc.gpsimd.memset(warm_i[:], 0.0)
            warm_o = cpool.tile([P, 1], F32, tag="warm_o")
            nc.scalar.activation(warm_o[:], warm_i[:], A.Tanh)

            wd_v = wd_sb[:].rearrange("p (j m) -> p j m", j=KE)

            def emit_tail(st):
                """Combine + store for a finished 512-half (one unit behind)."""
                h_len, off, ps_th, ps_cs, qacc, q0, tc_t = st
                if tail_on_pool:
                    # GPSIMD cannot read PSUM (verifier NCC): the PSUM-
                    # consuming ops stay on DVE; the SBUF-only final
                    # combine runs on the idle gpsimd engine.
                    p1 = tlpool.tile([P, H], F32, tag="p1")
                    nc.vector._custom_dve(
                        TAIL1,
                        out=p1[:, 0:h_len],
                        in0=tc_t[:, 0:h_len],
                        in1=ps_th[:, 0:h_len],
                        s0=c_sb[:, 2:3],
                        s1=c_sb[:, 3:4],
                    )
                    if k_act > 0:
                        p2 = tlpool.tile([P, H], F32, tag="p2")
                        nc.vector.scalar_tensor_tensor(
                            p2[:, 0:h_len], ps_cs[:, 0:h_len], c_sb[:, 4:5],
                            p1[:, 0:h_len], Op.mult, Op.add,
                        )
                    else:
                        p2 = p1
                    o_t = opool.tile([P, H], F16, tag="o")
                    nc.gpsimd.scalar_tensor_tensor(
                        o_t[:, 0:h_len], qacc[:, q0 : q0 + h_len],
                        c_sb[:, 5:6], p2[:, 0:h_len], Op.mult, Op.add,
                    )
                else:
                    # p1 = (tc+1) * (c_thm*S_th + c_tha)
                    p1 = tlpool.tile([P, H], F32, tag="p1")
                    nc.vector._custom_dve(
                        TAIL1,
                        out=p1[:, 0:h_len],
                        in0=tc_t[:, 0:h_len],
                        in1=ps_th[:, 0:h_len],
                        s0=c_sb[:, 2:3],
                        s1=c_sb[:, 3:4],
                    )
                    if k_act > 0:
                        # p2 = c_cs*S_cs + p1
                        p2 = tlpool.tile([P, H], F32, tag="p2")
                        nc.vector.scalar_tensor_tensor(
                            p2[:, 0:h_len], ps_cs[:, 0:h_len], c_sb[:, 4:5],
                            p1[:, 0:h_len], Op.mult, Op.add,
                        )
                    else:
                        p2 = p1
                    # out = c_q*qacc + p2   (fp16 out; c_0 rides the qacc seed)
                    o_t = opool.tile([P, H], F16, tag="o")
                    nc.vector.scalar_tensor_tensor(
                        o_t[:, 0:h_len], qacc[:, q0 : q0 + h_len],
                        c_sb[:, 5:6], p2[:, 0:h_len], Op.mult, Op.add,
                    )
                nc.sync.dma_start(o_d[:, off : off + h_len], o_t[:, 0:h_len])

            pending = None
            off = 0
            for t_idx, t_tile in enumerate(tiles):
                ft = KE * t_tile
                e0 = off * KE
                if t_idx == 0:
                    x_t = x_first
                else:
                    x_t = xpool.tile([P, ft_max], F16, tag="x")
                    nc.sync.dma_start(x_t[:, 0:ft], x_d[:, e0 : e0 + ft])

                # th = tanh((a/2) x) over all KE j-blocks
                th_t = thpool.tile([P, ft_max], F16, tag="th")
                nc.scalar.activation(
                    th_t[:, 0:ft], x_t[:, 0:ft], A.Tanh, scale=c_sb[:, 0:1]
                )
                # cs = sin(pi/2 - pi x) = cos(pi x) on the last k_act blocks
                if k_act > 0:
                    cs_t = cspool.tile([P, k_act * t_max], F16, tag="cs")
                    nc.scalar.activation(
                        cs_t[:, 0 : k_act * t_tile],
                        x_t[:, k_dve * t_tile : ft],
                        A.Sin,
                        bias=c_sb[:, 7:8],
                        scale=-PI,
                    )

                # Fused DVE sin^2 accumulation over j-blocks 0..k_dve-1;
                # the qacc seed carries the tail's additive constant c_0/c_q.
                qacc = qpool.tile([P, t_max], F32, tag="qacc")
                nc.vector._custom_dve(
                    SINSQ_INITB,
                    out=qacc[:, 0:t_tile],
                    in0=x_t[:, 0:t_tile],
                    in1=c_sb[:, 6:7],
                    s0=SIN_B2, s1=SIN_B1, imm2=SIN_B0,
                )
                for j in range(1, k_dve):
                    nc.vector._custom_dve(
                        SINSQ_ACC,
                        out=qacc[:, 0:t_tile],
                        in0=x_t[:, bass.ts(j, t_tile)],
                        in1=qacc[:, 0:t_tile],
                        s0=SIN_B2, s1=SIN_B1, imm2=SIN_B0,
                    )

                # PE reductions + tc + tail per 512-column half (PSUM bank)
                for h0 in range(0, t_tile, H):
                    h_len = min(H, t_tile - h0)
                    ps_wx = wxpool.tile([P, H], F32, tag="pwx")
                    ps_th = thppool.tile([P, H], F32, tag="pth")
                    if k_act:
                        ps_cs = csppool.tile([P, H], F32, tag="pcs")
                    else:
                        ps_cs = None

                    for j in range(KE):
                        nc.tensor.matmul(
                            ps_wx[:, 0:h_len],
                            lhsT=wd_v[:, j, :],
                            rhs=x_t[:, j * t_tile + h0 : j * t_tile + h0 + h_len],
                            start=(j == 0),
                            stop=(j == KE - 1),
                        )
                    for j in range(KE):
                        nc.tensor.matmul(
                            ps_th[:, 0:h_len],
                            lhsT=id_sb[:],
                            rhs=th_t[:, j * t_tile + h0 : j * t_tile + h0 + h_len],
                            start=(j == 0),
                            stop=(j == KE - 1),
                        )
                    for j in range(k_act):
                        nc.tensor.matmul(
                            ps_cs[:, 0:h_len],
                            lhsT=id_sb[:],
                            rhs=cs_t[:, j * t_tile + h0 : j * t_tile + h0 + h_len],
                            start=(j == 0),
                            stop=(j == k_act - 1),
                        )

                    # tc = tanh(0.5*S_wx + b/2)
                    tc_t = tcpool.tile([P, H], F32, tag="tc")
                    nc.scalar.activation(
                        tc_t[:, 0:h_len], ps_wx[:, 0:h_len], A.Tanh,
                        bias=c_sb[:, 1:2], scale=0.5,
                    )
                    if pending is not None:
                        emit_tail(pending)
                    pending = (h_len, off + h0, ps_th, ps_cs, qacc, h0, tc_t)
                off += t_tile
            emit_tail(pending)

    # Pin Tanh+Sin to the one table set holding both (silu_and_others) so
    # the act-table pass emits a single load instead of per-tile switches.
    import concourse.bacc as bacc
    import concourse.hw_specs as hw_specs

    _orig_gat = hw_specs.get_activation_tables

    def _pinned_tables(arch):
        tabs = {k: set(v) for k, v in _orig_gat(arch).items()}
        assert A.Tanh in tabs["silu_and_others"] and A.Sin in tabs["silu_and_others"]
        for name, fns in tabs.items():
            if name != "silu_and_others":
                fns.discard(A.Tanh)
                fns.discard(A.Sin)
        return tabs

    bacc.get_activation_tables = _pinned_tables
    try:
        nc.compile()
    finally:
        bacc.get_activation_tables = _orig_gat
    return nc


def get_nc(spp=SPP, tiles=None):
    if tiles is None:
        tiles = TILES
    key = (spp, tuple(tiles))
    if key not in _NC_CACHE:
        _NC_CACHE[key] = _build(spp, tiles)
    return _NC_CACHE[key]


def make_const_inputs(conv_w, conv_b, attn_w, alpha):
    """Host-side packing of the tiny runtime parameters."""
    w = np.asarray(conv_w, dtype=np.float32).reshape(KE)
    b = float(np.asarray(conv_b, np.float32).reshape(-1)[0])
    a = float(np.asarray(attn_w, np.float32).reshape(-1)[0])
    al = float(np.asarray(alpha, np.float32))

    wdiag = np.zeros((P, KE, P), dtype=np.float16)
    idx = np.arange(P)
    wdiag[idx, :, idx] = w[None, :].astype(np.float16)
    wdiag = np.ascontiguousarray(wdiag.reshape(P, KE * P))

    ident = np.ascontiguousarray(np.eye(P, dtype=np.float16))

    row = np.zeros(12, dtype=np.float32)
    row[0] = a / 2.0  # scale for tanh(a x / 2)
    row[1] = b / 2.0  # bias for tanh(0.5 S_wx + b/2)
    row[2] = al / 64.0  # TAIL1 c0 (S_th coeff)
    row[3] = al / 4.0  # TAIL1 c1
    row[4] = -(1.0 - al) / 32.0  # S_cs coeff
    row[5] = (1.0 - al) / 16.0  # qacc coeff
    row[6] = K_ACT / 2.0  # qacc seed: tail const (1-al)/32*K_ACT over row[5]
    row[7] = np.pi / 2.0  # bias for sin(pi/2 - pi x)
    consts = np.ascontiguousarray(np.tile(row[None, :], (P, 1)))
    return wdiag, ident, consts


def pack_x(x3d, tiles):
    """[..., spp, KE] f32 -> [..., spp*KE] fp16, tile-packed element-major."""
    *lead, spp, ke = x3d.shape
    assert sum(tiles) == spp
    v = x3d.astype(np.float16)
    out = np.empty((*lead, spp * ke), dtype=np.float16)
    off = 0
    for t in tiles:
        seg = np.swapaxes(v[..., off : off + t, :], -1, -2)
        out[..., off * ke : (off + t) * ke] = seg.reshape(*lead, t * ke)
        off += t
    return out


def prep_x(x, tiles=None):
    """Cast the f32 input to fp16, shard and tile-pack (cores, P, spp*KE)."""
    if tiles is None:
        tiles = TILES
    x = np.asarray(x)
    assert x.size == B * KE
    return pack_x(x.reshape(N_CORES, P, SPP, KE), tiles)


def kernel(x, conv_w, conv_b, attn_w, alpha):
    from concourse.bass_utils import run_bass_kernel_spmd

    xs = prep_x(x)
    wdiag, ident, consts = make_const_inputs(conv_w, conv_b, attn_w, alpha)

    nc = get_nc()
    in_maps = [
        {"x": xs[c], "wdiag": wdiag, "ident": ident, "consts": consts}
        for c in range(N_CORES)
    ]
    res = run_bass_kernel_spmd(nc, in_maps, list(range(N_CORES)))
    out = np.concatenate(
        [
            np.asarray(res.results[c]["out"]).astype(np.float32).reshape(-1)
            for c in range(N_CORES)
        ]
    )
    return out
